# revision 1
# baseline (speedup 1.0000x reference)
"""Multi-head attention kernel for Trainium2 (8 NeuronCores, SPMD).

Problem: x [4,1,2048,3], W_query/W_key/W_value [1,8,3,3] ->
ctx [4,8,2048,3] = softmax((x Wq)(x Wk)^T / sqrt(3)) @ (x Wv), returned
as a (ctx, ctx) tuple matching the reference.

Sharding: 32 (batch, head) blocks over 8 cores -> core c owns batch c//2,
heads 4*(c%2) .. +4. Each core runs an identical Bass program on its slice.

Per-core device program (S=2048, 4 heads split into 2 pairs):
  - All matmuls use float32r (fp32 with 11-bit mantissa, 1 cycle/column on
    the PE) with hi/lo split operands stacked along the contract dim, which
    recovers full fp32 precision: a*b = ah*bh + al*bh + ah*bl + al*bl with
    {ah,al,bh,bl} exact f32r pairs -> contract dim 3 becomes 12 (of 128).
  - x and the tiny weights are hi/lo split exactly on the host.
  - Q^T/K^T per head land in PSUM at col-group 32h, get hi/lo split on
    device (DVE round-copy + subtract) and are assembled into 12-row
    stacks per 32-partition row group via SBUF->SBUF DMAs.
  - Main loop per (head pair, 512-query chunk, 128-key tile):
      PE:  scores^T [k=128, q=512] per head, two heads concurrently in
           different PE row groups, into adjacent PSUM banks
      ACT: exp(scale * scores) over both banks in one instruction,
           writing f32r P tiles to SBUF
      PE:  [V | ones]^T @ P accumulated over key tiles in PSUM ->
           [ctx^T; denom] [4, 512] per head
  - Normalize: PE-transpose [4,128] -> [128,4], DVE reciprocal of the
    denominator lane + per-partition scalar multiply, DMA out.
"""

import math

import numpy as np

import concourse.bass as bass
import concourse.bacc as bacc
import concourse.tile as tile
from concourse import mybir
from concourse.bass_utils import run_bass_kernel_spmd

f32 = mybir.dt.float32
f32r = mybir.dt.float32r
bf16 = mybir.dt.bfloat16
EXP = mybir.ActivationFunctionType.Exp

B, H, S, D = 4, 8, 2048, 3
NCORES = 8
HPC = H // 2           # heads per core = 4
QCH = 512              # query chunk
NQ = S // QCH          # 4
KT = 128               # key tile
NKT = S // KT          # 16
SCALE = 1.0 / math.sqrt(D)


def _split_hi_lo(a: np.ndarray):
    """Exact split a = hi + lo with both parts f32r-representable
    (11-bit mantissa, round-to-nearest with carry)."""
    a = np.ascontiguousarray(a, dtype=np.float32)
    u = a.view(np.uint32)
    r = (u + np.uint32(0x7FF) + ((u >> np.uint32(12)) & np.uint32(1))) & np.uint32(
        0xFFFFF000
    )
    hi = r.view(np.float32)
    lo = (a - hi).astype(np.float32)
    return hi, lo


def _stack12(a: np.ndarray, pattern: str):
    """Stack hi/lo parts of a [3, N] array into [12, N] rows.
    pattern 'hlhl' -> [hi;lo;hi;lo], 'hhll' -> [hi;hi;lo;lo]."""
    hi, lo = _split_hi_lo(a)
    parts = {"h": hi, "l": lo}
    return np.concatenate([parts[p] for p in pattern], axis=0)


def _build_nc():
    nc = bacc.Bacc("TRN2", target_bir_lowering=False, debug=False,
                   num_devices=NCORES)

    xstk_in = nc.dram_tensor("xstk", [12, S], f32r, kind="ExternalInput").ap()
    wq_in = nc.dram_tensor("wqstk", [12, 12], f32r, kind="ExternalInput").ap()
    wk_in = nc.dram_tensor("wkstk", [12, 12], f32r, kind="ExternalInput").ap()
    xo_in = nc.dram_tensor("xo", [128, NKT, 7], f32r, kind="ExternalInput").ap()
    wv7_in = nc.dram_tensor("wv7", [7, 16], f32, kind="ExternalInput").ap()
    out = nc.dram_tensor("out", [HPC, S, D], f32, kind="ExternalOutput").ap()

    with tile.TileContext(nc) as tc:
        with tc.tile_pool(name="persist", bufs=1) as per, \
             tc.tile_pool(name="work", bufs=1) as work:
            # ---------------- setup ----------------
            xstk = per.tile([128, S], f32r)
            wq = per.tile([128, 12], f32r)
            wk = per.tile([128, 12], f32r)
            xo = per.tile([128, NKT, 7], f32r)
            wv7 = per.tile([128, 16], f32)
            nc.sync.dma_start(out=xstk[0:12, :], in_=xstk_in)
            nc.sync.dma_start(out=wq[0:12, :], in_=wq_in)
            nc.sync.dma_start(out=wk[0:12, :], in_=wk_in)
            nc.sync.dma_start(out=xo, in_=xo_in)
            nc.sync.dma_start(out=wv7[0:7, :], in_=wv7_in)

            qstk = per.tile([128, S], bf16)
            kstk = per.tile([128, S], bf16)
            # zero-fill: rows 18-31 of each group are padding so the QK
            # matmuls can use full 32-row groups (18-row matmuls leave the
            # PE's activity monitor cold and the clock stays at 1.2 GHz)
            nc.vector.memset(qstk, 0.0)
            nc.vector.memset(kstk, 0.0)

            with tc.tile_pool(name="setup_sb", bufs=1) as ssb, \
                 tc.tile_pool(name="setup_ps", bufs=2, space="PSUM") as sps:
                # Q/K projections: one f32r matmul per (tensor, 512-chunk)
                # producing all 4 heads as output rows 3h+e at base 0
                # (f32r matmuls require dst partition base 0), then a
                # 3-way bf16 split straight from PSUM:
                #   a1 = bf16(a); a2 = bf16(a-a1); a3 = bf16(a-a1-a2)
                # Stacks are assembled per 512-chunk so the main loop's
                # first chunk starts as soon as chunk 0 is ready.
                # term t6 of q.k: sum_d Qrow[t6]*Krow[t6]:
                #   Q rows [q1;q1;q2;q1;q3;q2], K rows [k1;k2;k1;k3;k1;k2]
                q_order = (0, 0, 1, 0, 2, 1)
                k_order = (0, 1, 0, 2, 0, 1)
                qparts = [ssb.tile([128, S], bf16, name=f"q{i}") for i in (1, 2, 3)]
                kparts = [ssb.tile([128, S], bf16, name=f"k{i}") for i in (1, 2, 3)]
                for qc in range(NQ):
                    cs = slice(qc * QCH, (qc + 1) * QCH)
                    for nm, w_sb, parts in (
                        ("q", wq, qparts),
                        ("k", wk, kparts),
                    ):
                        pj = sps.tile([128, QCH], f32, name=f"pj{nm}{qc}",
                                      tag=f"pj{nm}")
                        nc.tensor.matmul(
                            pj[0:12, :],
                            lhsT=w_sb[0:12, 0:12],
                            rhs=xstk[0:12, cs],
                            start=True, stop=True,
                        )
                        a1, a2, a3 = parts
                        rem = ssb.tile([128, QCH], f32, name=f"rem{nm}{qc}",
                                       tag="rem", bufs=2)
                        # round-copies on ACT, subtractions on DVE so the
                        # two engines pipeline the 4-op chain
                        nc.scalar.copy(a1[0:12, cs], pj[0:12, :])
                        nc.vector.tensor_sub(rem[0:12, :], pj[0:12, :],
                                             a1[0:12, cs])
                        nc.scalar.copy(a2[0:12, cs], rem[0:12, :])
                        nc.vector.tensor_sub(a3[0:12, cs], rem[0:12, :],
                                             a2[0:12, cs])
                # whole-row stack DMAs after all chunks (per-chunk DMAs
                # would be write-after-read hazards against live QK reads).
                # Groups 0/1 (first head pair) before 2/3, source-part-major
                # within that; spread across the sync HWDGE queue and the
                # gpsimd SWDGE queue (never the scalar queue - DMA triggers
                # block the ACT engine, the kernel's bottleneck).
                dma_engines = (nc.sync, nc.gpsimd)
                di = 0
                for gg in ((0, 1), (2, 3)):
                    for part_idx in range(3):
                        for g in gg:
                            for t6 in range(6):
                                r = 32 * g + 3 * t6
                                if q_order[t6] == part_idx:
                                    dma_engines[di % 2].dma_start(
                                        out=qstk[r:r + 3, :],
                                        in_=qparts[part_idx][3 * g:3 * g + 3, :])
                                    di += 1
                                if k_order[t6] == part_idx:
                                    dma_engines[di % 2].dma_start(
                                        out=kstk[r:r + 3, :],
                                        in_=kparts[part_idx][3 * g:3 * g + 3, :])
                                    di += 1

            # ---------------- main loop (software-pipelined emission) ----
            # Per key tile t the chain is QK(t) -> exp(t) -> PV(t); emitting
            # in that order serializes the PE stream (each matmul pays full
            # isolated-MM latency waiting on ACT). Emit QK(t+1) before PV(t)
            # so the PE always has independent work, and drip the previous
            # chunk's normalization ops one piece per key tile.
            with tc.tile_pool(name="s_ps", bufs=2, space="PSUM") as spsum, \
                 tc.tile_pool(name="c_ps", bufs=1, space="PSUM") as cpsum, \
                 tc.tile_pool(name="t_ps", bufs=2, space="PSUM") as tpsum:
                pending = []          # deferred normalize pieces (closures)

                def drain(n=1):
                    for _ in range(n):
                        if pending:
                            pending.pop(0)()

                for pair in range(2):
                    heads = (2 * pair, 2 * pair + 1)
                    for qc in range(NQ):
                        cs = slice(qc * QCH, (qc + 1) * QCH)
                        ctx_ps = [
                            cpsum.tile([128, QCH], f32,
                                       name=f"ctx{pair}{qc}{jj}", tag=f"ctx{jj}")
                            for jj in range(2)
                        ]

                        def emit_qk(t, _pair=pair, _qc=qc, _cs=cs,
                                    _heads=heads):
                            s_ps = spsum.tile([128, 2 * QCH], f32,
                                              name=f"s{_pair}{_qc}{t}", tag="s")
                            for jj, h in enumerate(_heads):
                                g = 32 * h
                                nc.tensor.matmul(
                                    s_ps[:, jj * QCH:(jj + 1) * QCH],
                                    lhsT=kstk[g:g + 32, t * KT:(t + 1) * KT],
                                    rhs=qstk[g:g + 32, _cs],
                                    start=True, stop=True,
                                    tile_position=(g, 0),
                                )
                            return s_ps

                        s_cur = emit_qk(0)
                        for t in range(NKT):
                            drain(2 if t == 0 else 1)
                            p_sb = work.tile([128, 2 * QCH], f32r,
                                             name=f"p{pair}{qc}{t}", tag="p",
                                             bufs=3)
                            nc.scalar.activation(p_sb, s_cur, EXP, scale=SCALE)
                            if t + 1 < NKT:
                                s_cur = emit_qk(t + 1)
                            for jj, h in enumerate(heads):
                                nc.tensor.matmul(
                                    ctx_ps[jj][0:7, :],
                                    lhsT=xo[:, t, :],
                                    rhs=p_sb[:, jj * QCH:(jj + 1) * QCH],
                                    start=(t == 0), stop=(t == NKT - 1),
                                )

                        # queue this chunk's normalization as pieces
                        ostage = work.tile([128, 2, 4, 3], f32,
                                           name=f"o{pair}{qc}", tag="ostage",
                                           bufs=2)
                        ctx_sbs = [
                            work.tile([128, QCH], f32, name=f"cs{pair}{qc}{jj}",
                                      tag=f"ctx_sb{jj}", bufs=2)
                            for jj in range(2)
                        ]

                        def mk_copy(jj, _ctx=ctx_ps, _sb=ctx_sbs):
                            def go():
                                nc.vector.tensor_copy(_sb[jj][0:7, :],
                                                      _ctx[jj][0:7, :])
                            return go

                        def mk_piece(jj, c4, _pair=pair, _qc=qc, _h=None,
                                     _sb=ctx_sbs, _ost=ostage):
                            h4 = 4 * (2 * _pair + jj)
                            def go():
                                # fused transpose + Wv contraction + denom:
                                # ct[q, e] = sum_d U7[d, q] * wv7[d, 4h+e]
                                ct = tpsum.tile(
                                    [128, 4], f32,
                                    name=f"ct{_pair}{_qc}{jj}{c4}", tag="ct")
                                nc.tensor.matmul(
                                    ct,
                                    lhsT=_sb[jj][0:7, c4 * 128:(c4 + 1) * 128],
                                    rhs=wv7[0:7, h4:h4 + 4],
                                    start=True, stop=True,
                                )
                                rec = work.tile(
                                    [128, 1], f32,
                                    name=f"r{_pair}{_qc}{jj}{c4}",
                                    tag="rec", bufs=4)
                                nc.vector.reciprocal(rec, ct[:, 3:4])
                                nc.vector.tensor_scalar_mul(
                                    _ost[:, jj, c4, :], ct[:, 0:3], rec)
                            return go

                        def mk_out(jj, _pair=pair, _qc=qc, _ost=ostage):
                            def go():
                                dst = bass.AP(
                                    tensor=out.tensor,
                                    offset=((2 * _pair + jj) * S * D
                                            + _qc * QCH * D),
                                    ap=[[D, 128], [128 * D, 4], [1, D]],
                                )
                                nc.sync.dma_start(out=dst,
                                                  in_=_ost[:, jj, :, :])
                            return go

                        pending += [mk_copy(0), mk_copy(1)]
                        pending += [mk_piece(jj, c4)
                                    for jj in range(2) for c4 in range(4)]
                        pending += [mk_out(0), mk_out(1)]
                drain(len(pending))

    nc.compile()
    return nc


_NC_CACHE = None


def _get_nc():
    global _NC_CACHE
    if _NC_CACHE is None:
        _NC_CACHE = _build_nc()
    return _NC_CACHE


def _make_in_maps(x, W_query, W_key, W_value):
    in_maps = []
    for c in range(NCORES):
        b = c // 2
        hp = (c % 2) * HPC
        xb = x[b, 0]                                    # [S, 3]
        xT = np.ascontiguousarray(xb.T)                 # [3, S]
        xstk = _stack12(xT, "hlhl")                     # [12, S]

        def wstack(W):
            # [3(d), 12(3h+e)] column layout, then rows [wh;wh;wl;wl]
            wt = np.ascontiguousarray(
                W[0, hp:hp + HPC].transpose(1, 0, 2).reshape(3, 12))
            return _stack12(wt, "hhll")

        # xo[p, t, :] = [x_hi(3) | x_lo(3) | 1] at position t*128+p
        xh, xl = _split_hi_lo(xb)
        xo = np.concatenate([xh, xl, np.ones((S, 1), np.float32)], axis=1)
        xo = np.ascontiguousarray(xo.reshape(NKT, 128, 7).transpose(1, 0, 2))

        # wv7[:, 4h+e] = [Wv_h[:, e]; Wv_h[:, e]; 0], wv7[:, 4h+3] = e_denom
        wv7 = np.zeros((7, 16), np.float32)
        for h in range(HPC):
            Wv = W_value[0, hp + h]                     # [3, 3]
            wv7[0:3, 4 * h:4 * h + 3] = Wv
            wv7[3:6, 4 * h:4 * h + 3] = Wv
            wv7[6, 4 * h + 3] = 1.0

        in_maps.append({
            "xstk": xstk,
            "wqstk": wstack(W_query),
            "wkstk": wstack(W_key),
            "xo": xo,
            "wv7": wv7,
        })
    return in_maps


def kernel(x, W_query, W_key, W_value, _trace=False, _tmpdir=None):
    x = np.asarray(x, dtype=np.float32)
    W_query = np.asarray(W_query, dtype=np.float32)
    W_key = np.asarray(W_key, dtype=np.float32)
    W_value = np.asarray(W_value, dtype=np.float32)

    nc = _get_nc()
    res = run_bass_kernel_spmd(
        nc,
        _make_in_maps(x, W_query, W_key, W_value),
        core_ids=list(range(NCORES)),
        trace=_trace,
        tmpdir=_tmpdir,
    )
    full = np.empty((B, H, S, D), dtype=np.float32)
    for c in range(NCORES):
        b = c // 2
        hp = (c % 2) * HPC
        full[b, hp:hp + HPC] = res.results[c]["out"]
    if _trace:
        kernel._last_results = res
    return (full, full)

